# revision 1
# baseline (speedup 1.0000x reference)
"""Self-contained Trainium2 Bass kernel for the 2-layer GAT (nn_GAT_6451040878848).

Sharding: nodes are permuted by (quantized in-degree, low-bank neighbor count)
and dealt round-robin to 8 cores; core k owns a contiguous `per_core`-row
octant of the permuted node table and aggregates every edge whose dst is in its
octant (dst-octant edge sharding -> no all-reduce). The only collective is one
bf16 AllGather of the layer-2 node table. Layer-1 node phase is replicated
from a host-transposed bf16 copy of x.

Edge phase: per 128-dst tile, edges are slot-major (slot c = c-th in-edge of
each dst; dst == partition), gathered from the node table with dma_gather
(int16 idx, two 65536-row banks via biased bases), weights w =
exp(leaky_relu(alpha_s[src]+alpha_d[dst])) (no max-subtraction: logits are in
[-1,3]), and scattered by an identity-lhsT matmul accumulating in PSUM; the
softmax denominator rides along as extra rhs columns.
"""
import numpy as np
import ml_dtypes

import concourse.bacc as bacc
import concourse.bass as bass
import concourse.tile as tile
from concourse import mybir
from concourse.bass_utils import run_bass_kernel_spmd

P = 128
TROW = 128           # bf16 elements per node-table row (256B)
NEG_SLOPE = 0.2
F_IN = 128
H1, C1 = 8, 8
C2 = 64
BANK_ROWS = 65536
DEG_QUANT = 4
NCORES = 8

bf16 = ml_dtypes.bfloat16


# ----------------------------------------------------------------------------
# Host-side graph preprocessing (integer/index work only)
# ----------------------------------------------------------------------------
def host_prep(edge_index: np.ndarray, n_nodes: int, n_cores: int = NCORES):
    N = n_nodes
    loops = np.arange(N, dtype=np.int64)
    src = np.concatenate([edge_index[0].astype(np.int64), loops])
    dst = np.concatenate([edge_index[1].astype(np.int64), loops])

    deg = np.bincount(dst, minlength=N)
    degq = -(-deg // DEG_QUANT) * DEG_QUANT

    per_core = -(-N // (n_cores * P)) * P          # 128-aligned octant size
    Npad = per_core * n_cores
    n_tiles = per_core // P
    rows_total = Npad + 3                          # 0=padlow, 1..Npad nodes(+dummies), Npad+1=padhigh
    single_bank = rows_total <= 32768

    if single_bank:
        aprov = np.zeros(N, dtype=np.int64)
    else:
        rank0 = np.argsort(degq, kind="stable")
        pos0 = np.empty(N, dtype=np.int64)
        pos0[rank0] = (rank0 % n_cores) * 0        # noqa (placeholder)
        core0 = np.empty(N, dtype=np.int64); local0 = np.empty(N, dtype=np.int64)
        core0[rank0] = np.arange(N) % n_cores
        local0[rank0] = np.arange(N) // n_cores
        pos0 = core0 * per_core + local0
        lowbank0 = (1 + pos0) < BANK_ROWS
        aprov = np.bincount(dst[lowbank0[src]], minlength=N)

    rank = np.lexsort((aprov, degq))
    node_rank = np.empty(N, dtype=np.int64)
    node_rank[rank] = np.arange(N)
    core_of = node_rank % n_cores
    local_of = node_rank // n_cores
    pos = core_of * per_core + local_of
    row = 1 + pos
    node_of_pos = np.full(Npad, -1, dtype=np.int64)
    node_of_pos[pos] = np.arange(N)

    lowbank = row < BANK_ROWS
    pad_low_row = 0
    pad_high_row = Npad + 1
    base_low = 0 if single_bank else BANK_ROWS // 2
    base_high = 0 if single_bank else (rows_total - BANK_ROWS + BANK_ROWS // 2)
    if not single_bank:
        assert rows_total <= 2 * BANK_ROWS, "need more than 2 banks"

    e_core = core_of[dst]
    e_tile = local_of[dst] // P
    e_part = local_of[dst] % P
    e_low = lowbank[src] if not single_bank else np.ones(len(src), bool)
    e_srcrow = row[src]

    cnt_low = np.zeros((n_cores, n_tiles, P), dtype=np.int32)
    cnt_high = np.zeros((n_cores, n_tiles, P), dtype=np.int32)
    np.add.at(cnt_low, (e_core[e_low], e_tile[e_low], e_part[e_low]), 1)
    if not single_bank:
        np.add.at(cnt_high, (e_core[~e_low], e_tile[~e_low], e_part[~e_low]), 1)

    D_low = cnt_low.max(axis=(0, 2)).astype(np.int64)
    D_high = cnt_high.max(axis=(0, 2)).astype(np.int64)
    # +1 throwaway slot terminating the low bank's idx list with a positive idx
    extra = 0 if single_bank else 1
    slots_low = D_low + extra

    # ---- slot assignment ----
    order = np.lexsort((np.where(e_low, 0, 1), e_part, e_tile, e_core))
    so_core = e_core[order]; so_tile = e_tile[order]; so_part = e_part[order]
    so_low = e_low[order]; so_row = e_srcrow[order]
    grp = ((so_core * n_tiles + so_tile) * P + so_part) * 2 + np.where(so_low, 0, 1)
    _, inv, counts = np.unique(grp, return_inverse=True, return_counts=True)
    starts = np.concatenate([[0], np.cumsum(counts)[:-1]])
    slot_in_bank = np.arange(len(grp)) - starts[inv]

    max_slots = int((slots_low + D_high).max()) if n_tiles else 0
    srows = np.full((n_cores, n_tiles, max_slots, P), -1, dtype=np.int64)
    slot_abs = np.where(so_low, slot_in_bank, slots_low[so_tile] + slot_in_bank)
    srows[so_core, so_tile, slot_abs, so_part] = so_row

    for t in range(n_tiles):
        dl = int(D_low[t]); sl = int(slots_low[t]); dh = int(D_high[t])
        blk = srows[:, t, :dl, :]
        blk[blk < 0] = pad_low_row
        srows[:, t, :dl, :] = blk
        if not single_bank:
            srows[:, t, dl, :] = base_low          # throwaway slot (idx 0, positive)
            if dh:
                blk2 = srows[:, t, sl:sl + dh, :]
                blk2[blk2 < 0] = pad_high_row
                srows[:, t, sl:sl + dh, :] = blk2

    # ---- int16 idx arrays ----
    col_off = []
    cols = 0
    for t in range(n_tiles):
        lo = cols; cols += int(slots_low[t]) * 8
        hi = cols; cols += int(D_high[t]) * 8
        col_off.append((lo, hi))
    idx16 = np.zeros((n_cores, 16, max(cols, 1)), dtype=np.int16)
    for t in range(n_tiles):
        sl = int(slots_low[t]); dh = int(D_high[t])
        lo, hi = col_off[t]
        jl = srows[:, t, :sl, :].reshape(n_cores, -1) - base_low
        assert jl.max() <= 32767 and jl.min() >= -32768
        idx16[:, :, lo:lo + sl * 8] = jl.astype(np.int16).reshape(n_cores, -1, 16).transpose(0, 2, 1)
        if dh:
            jh = srows[:, t, sl:sl + dh, :].reshape(n_cores, -1) - base_high
            assert jh.max() <= 32767 and jh.min() >= -32768
            idx16[:, :, hi:hi + dh * 8] = jh.astype(np.int16).reshape(n_cores, -1, 16).transpose(0, 2, 1)
    idx16 = np.tile(idx16, (1, 8, 1))

    meta = dict(
        N=N, Npad=Npad, n_cores=n_cores, per_core=per_core, n_tiles=n_tiles,
        rows_total=rows_total, single_bank=single_bank,
        base_low=int(base_low), base_high=int(base_high),
        slots_low=[int(v) for v in slots_low],
        D_low=[int(v) for v in D_low],
        D_high=[int(v) for v in D_high],
        col_off=col_off, idx_cols=int(max(cols, 1)),
    )
    return meta, idx16, node_of_pos


# ----------------------------------------------------------------------------
# Device kernel
# ----------------------------------------------------------------------------
def build_kernel(meta):
    Npad = meta["Npad"]; n_cores = meta["n_cores"]; per_core = meta["per_core"]
    n_tiles = meta["n_tiles"]; rows_total = meta["rows_total"]
    slots_low = meta["slots_low"]; D_low = meta["D_low"]; D_high = meta["D_high"]
    col_off = meta["col_off"]; idx_cols = meta["idx_cols"]
    single_bank = meta["single_bank"]
    base_low = meta["base_low"]; base_high = meta["base_high"]
    n_groups = Npad // (4 * P)

    nc = bacc.Bacc("TRN2", target_bir_lowering=False, debug=False,
                   num_devices=n_cores, num_swdge_queues=4)
    f32, b16, i16 = mybir.dt.float32, mybir.dt.bfloat16, mybir.dt.int16
    AF = mybir.ActivationFunctionType
    OP = mybir.AluOpType

    xT = nc.dram_tensor("xT", [F_IN, Npad], b16, kind="ExternalInput").ap()
    W1 = nc.dram_tensor("W1", [F_IN, 64], f32, kind="ExternalInput").ap()
    W1T = nc.dram_tensor("W1T", [64, F_IN], f32, kind="ExternalInput").ap()
    A1 = nc.dram_tensor("A1", [64, 16], f32, kind="ExternalInput").ap()
    W2 = nc.dram_tensor("W2", [64, C2], f32, kind="ExternalInput").ap()
    W2T = nc.dram_tensor("W2T", [C2, 64], f32, kind="ExternalInput").ap()
    A2 = nc.dram_tensor("A2", [C2, 2], f32, kind="ExternalInput").ap()
    B1 = nc.dram_tensor("B1", [1, 64], f32, kind="ExternalInput").ap()
    B2 = nc.dram_tensor("B2", [1, C2], f32, kind="ExternalInput").ap()
    IDX = nc.dram_tensor("IDX", [P, idx_cols], i16, kind="ExternalInput").ap()
    OUT = nc.dram_tensor("OUT", [per_core, C2], f32, kind="ExternalOutput").ap()

    with tile.TileContext(nc) as tc:
        with tc.tile_pool(name="dram", bufs=1, space="DRAM") as dram, \
             tc.tile_pool(name="consts", bufs=1) as cp, \
             tc.tile_pool(name="stg", bufs=3) as nsp, \
             tc.tile_pool(name="xtp", bufs=3) as xtp, \
             tc.tile_pool(name="gpl", bufs=3) as gp, \
             tc.tile_pool(name="idxp", bufs=3) as idxp, \
             tc.tile_pool(name="vwp", bufs=3) as vwp, \
             tc.tile_pool(name="stat", bufs=4) as sp, \
             tc.tile_pool(name="pacc", bufs=2, space="PSUM") as pacc, \
             tc.tile_pool(name="pnode", bufs=2, space="PSUM") as pnode, \
             tc.tile_pool(name="ptr", bufs=2, space="PSUM") as ptr, \
             tc.tile_pool(name="pl2", bufs=2, space="PSUM") as pl2:

            table1 = dram.tile([rows_total, TROW], b16)
            table2 = dram.tile([rows_total, TROW], b16)
            h2loc = dram.tile([per_core, TROW], b16)

            # ------------- constants -------------
            ident = cp.tile([P, P], b16)
            nc.gpsimd.memset(ident[:], 0.0)
            iota_i = cp.tile([P, 1], mybir.dt.int32)
            nc.gpsimd.iota(iota_i[:], pattern=[[0, 1]], base=0, channel_multiplier=1)
            iota_f = cp.tile([P, 1], f32)
            nc.vector.tensor_copy(out=iota_f[:], in_=iota_i[:])
            iotar_i = cp.tile([P, P], mybir.dt.int32)
            nc.gpsimd.iota(iotar_i[:], pattern=[[1, P]], base=0, channel_multiplier=0)
            iotar_f = cp.tile([P, P], f32)
            nc.vector.tensor_copy(out=iotar_f[:], in_=iotar_i[:])
            nc.vector.tensor_scalar(out=ident[:], in0=iotar_f[:], scalar1=iota_f[:],
                                    scalar2=None, op0=OP.is_equal)

            w1f = cp.tile([P, 64], f32)
            nc.sync.dma_start(out=w1f[:], in_=W1)
            w1t = cp.tile([64, P], f32)
            nc.sync.dma_start(out=w1t[:], in_=W1T)
            a1t = cp.tile([64, 16], f32)
            nc.sync.dma_start(out=a1t[:], in_=A1)
            w2f = cp.tile([64, 64], f32)
            nc.sync.dma_start(out=w2f[:], in_=W2)
            w2t = cp.tile([64, 64], f32)
            nc.sync.dma_start(out=w2t[:], in_=W2T)
            a2t = cp.tile([64, 2], f32)
            nc.sync.dma_start(out=a2t[:], in_=A2)
            b1r = cp.tile([1, 64], f32)
            nc.sync.dma_start(out=b1r[:], in_=B1)
            b1b = cp.tile([P, 64], f32)
            nc.gpsimd.partition_broadcast(b1b[:], b1r[:])
            b2r = cp.tile([1, 64], f32)
            nc.sync.dma_start(out=b2r[:], in_=B2)
            b2b = cp.tile([P, 64], f32)
            nc.gpsimd.partition_broadcast(b2b[:], b2r[:])

            wext1 = cp.tile([P, 80], b16)
            ws_ps = pnode.tile([P, 320], f32, space="PSUM", tag="np")
            nc.tensor.matmul(out=ws_ps[:, 0:16], lhsT=w1t[:], rhs=a1t[:], start=True, stop=True)
            nc.vector.tensor_copy(out=wext1[:, 0:64], in_=w1f[:])
            nc.vector.tensor_copy(out=wext1[:, 64:80], in_=ws_ps[:, 0:16])

            w2ext = cp.tile([64, 66], b16)
            ws2_ps = pnode.tile([P, 320], f32, space="PSUM", tag="np")
            nc.tensor.matmul(out=ws2_ps[:64, 0:2], lhsT=w2t[:], rhs=a2t[:], start=True, stop=True)
            nc.vector.tensor_copy(out=w2ext[:, 0:64], in_=w2f[:])
            nc.vector.tensor_copy(out=w2ext[:, 64:66], in_=ws2_ps[:64, 0:2])

            padrow = cp.tile([1, TROW], b16)
            nc.gpsimd.memset(padrow[:], 0.0)
            nc.gpsimd.memset(padrow[:, 64:72], -40.0)
            for tbl in (table1, table2):
                nc.sync.dma_start(out=tbl[0:1, :], in_=padrow[:])
                nc.sync.dma_start(out=tbl[Npad + 1:Npad + 2, :], in_=padrow[:])

            # core id register (for alpha-d SBUF slicing)
            pid = nc.partition_id()

            # ------------- L1 node phase (replicated) -------------
            for g in range(n_groups):
                base = g * 4 * P
                xt = xtp.tile([P, 4 * P], b16, tag="xt")
                nc.sync.dma_start(out=xt[:], in_=xT[:, base:base + 4 * P])
                ps = pnode.tile([P, 320], f32, space="PSUM", tag="np")
                stage = nsp.tile([P, 4 * 80], b16, tag="stage")
                for s in range(4):
                    nc.tensor.matmul(out=ps[:, s * 80:(s + 1) * 80],
                                     lhsT=xt[:, s * P:(s + 1) * P],
                                     rhs=wext1[:], start=True, stop=True)
                nc.scalar.activation(
                    out=stage[:], in_=ps[:], func=AF.Copy)
                nc.sync.dma_start(
                    out=table1[1 + base:1 + base + 4 * P, 0:80].rearrange("(s p) r -> p s r", p=P),
                    in_=stage[:].rearrange("p (s r) -> p s r", r=80))

            # ------------- alpha-d preload helper -------------
            def load_alpha_d(table, cols, width):
                """[128, n_cores * n_tiles * width] from table[row, 64+cols...]"""
                t_all = cp.tile([P, n_cores * n_tiles * width], b16,
                                tag=f"ad{cols}")
                for k in range(n_cores):
                    r0 = 1 + k * per_core
                    nc.sync.dma_start(
                        out=t_all[:, k * n_tiles * width:(k + 1) * n_tiles * width]
                            .rearrange("p (t w) -> p t w", w=width),
                        in_=table[r0:r0 + per_core, 72 + cols:72 + cols + width]
                            .rearrange("(t p) w -> p t w", p=P))
                return t_all

            # ------------- edge phase -------------
            def edge_phase(table, layer, ad_all, emit):
                """layer 1: heads=8, vw=72; layer 2: heads=1, vw=65."""
                heads = H1 if layer == 1 else 1
                vw_w = 72 if layer == 1 else 65
                if single_bank:
                    in_lo, in_hi = table[:, :], None
                else:
                    in_lo = table[base_low:min(base_low + BANK_ROWS, rows_total), :]
                    in_hi = table[base_high:rows_total, :]
                for t in range(n_tiles):
                    sl = slots_low[t]; dl = D_low[t]; dh = D_high[t]
                    stot = sl + dh
                    lo_c, hi_c = col_off[t]
                    idxt = idxp.tile([P, (sl + dh) * 8], i16, tag="idx")
                    nc.sync.dma_start(out=idxt[:], in_=IDX[:, lo_c:lo_c + (sl + dh) * 8])
                    G = gp.tile([P, stot * TROW], b16, tag="G")
                    nc.gpsimd.dma_gather(
                        out_ap=G[:, 0:sl * TROW].rearrange("p (s r) -> p s r", r=TROW),
                        in_ap=in_lo, idxs_ap=idxt[:, 0:sl * 8],
                        num_idxs=sl * P, num_idxs_reg=sl * P, elem_size=TROW,
                        queue_num=(2 * t) % 4, single_packet=False)
                    if dh:
                        nc.gpsimd.dma_gather(
                            out_ap=G[:, sl * TROW:stot * TROW].rearrange("p (s r) -> p s r", r=TROW),
                            in_ap=in_hi, idxs_ap=idxt[:, sl * 8:(sl + dh) * 8],
                            num_idxs=dh * P, num_idxs_reg=dh * P, elem_size=TROW,
                            queue_num=(2 * t + 1) % 4, single_packet=False)

                    # real slots: [0,dl) and [sl, sl+dh)
                    Gv = G[:].rearrange("p (s r) -> p s r", r=TROW)
                    adw = heads
                    ad_off = pid * (n_tiles * adw) + t * adw
                    ad_t = sp.tile([P, adw], b16, tag="adt")
                    nc.vector.tensor_copy(out=ad_t[:], in_=ad_all[:, bass.ds(ad_off, adw)])
                    w_all = sp.tile([P, stot * 8], f32, tag="wf")
                    wb_all = sp.tile([P, stot * 8], b16, tag="wb")
                    for (s0, ns) in ((0, dl), (sl, dh)) if dh else ((0, dl),):
                        if ns == 0:
                            continue
                        als = Gv[:, s0:s0 + ns, 64:64 + heads]
                        e_t = sp.tile([P, stot * 8], f32, tag="et")
                        if layer == 1:
                            adv = ad_t[:].unsqueeze(1).broadcast_to([P, ns, 8])
                            nc.vector.tensor_tensor(
                                out=e_t[:, s0 * 8:(s0 + ns) * 8].rearrange("p (s h) -> p s h", h=8),
                                in0=als, in1=adv, op=OP.add)
                            nc.scalar.activation(
                                out=e_t[:, s0 * 8:(s0 + ns) * 8],
                                in_=e_t[:, s0 * 8:(s0 + ns) * 8],
                                func=AF.Prelu, alpha=NEG_SLOPE)
                            nc.scalar.activation(
                                out=w_all[:, s0 * 8:(s0 + ns) * 8],
                                in_=e_t[:, s0 * 8:(s0 + ns) * 8], func=AF.Exp)
                            nc.vector.tensor_copy(
                                out=wb_all[:, s0 * 8:(s0 + ns) * 8],
                                in_=w_all[:, s0 * 8:(s0 + ns) * 8])
                        else:
                            adv = ad_t[:]
                            nc.scalar.activation(
                                out=e_t[:, s0:s0 + ns],
                                in_=als.rearrange("p s one -> p (s one)"),
                                func=AF.Prelu, bias=adv, alpha=NEG_SLOPE)
                            nc.scalar.activation(
                                out=w_all[:, s0:s0 + ns],
                                in_=e_t[:, s0:s0 + ns], func=AF.Exp)
                            nc.vector.tensor_copy(
                                out=wb_all[:, s0:s0 + ns],
                                in_=w_all[:, s0:s0 + ns])

                    Vw = vwp.tile([P, stot * vw_w], b16, tag="vw")
                    Vv = Vw[:].rearrange("p (s c) -> p s c", c=vw_w)
                    for (s0, ns) in ((0, dl), (sl, dh)) if dh else ((0, dl),):
                        if ns == 0:
                            continue
                        if layer == 1:
                            wbv = wb_all[:].rearrange("p (s h) -> p s h", h=8)[:, s0:s0 + ns, :] \
                                .unsqueeze(3).broadcast_to([P, ns, 8, 8])
                            nc.vector.tensor_tensor(
                                out=Vv[:, s0:s0 + ns, 0:64].rearrange("p s (h c) -> p s h c", c=8),
                                in0=Gv[:, s0:s0 + ns, 0:64].rearrange("p s (h c) -> p s h c", c=8),
                                in1=wbv, op=OP.mult)
                            nc.vector.tensor_copy(
                                out=Vv[:, s0:s0 + ns, 64:72],
                                in_=wb_all[:].rearrange("p (s h) -> p s h", h=8)[:, s0:s0 + ns, :])
                        else:
                            wbv = wb_all[:, s0:s0 + ns].unsqueeze(2).broadcast_to([P, ns, 64])
                            nc.vector.tensor_tensor(
                                out=Vv[:, s0:s0 + ns, 0:64],
                                in0=Gv[:, s0:s0 + ns, 0:64], in1=wbv, op=OP.mult)
                            nc.vector.tensor_copy(
                                out=Vv[:, s0:s0 + ns, 64:65],
                                in_=wb_all[:, s0:s0 + ns].unsqueeze(2))

                    acc = pacc.tile([P, vw_w], f32, space="PSUM", tag="acc")
                    real = [s for s in range(dl)] + [s for s in range(sl, stot)]
                    for i, s in enumerate(real):
                        nc.tensor.matmul(out=acc[:], lhsT=ident[:],
                                         rhs=Vw[:, s * vw_w:(s + 1) * vw_w],
                                         start=(i == 0), stop=(i == len(real) - 1))
                    emit(t, acc)

            # ------------- L1 -------------
            import os as _os2
            ad1_all = load_alpha_d(table1, 0, 8)
            out1 = cp.tile([P, n_tiles * 80], b16)

            def emit1(t, acc):
                nc.vector.tensor_copy(out=out1[:, t * 80:t * 80 + 64], in_=acc[:, 0:64])
                nc.vector.tensor_scalar(out=out1[:, t * 80 + 64:t * 80 + 72],
                                        in0=acc[:, 64:72], scalar1=1e-16,
                                        scalar2=None, op0=OP.add)

            if _os2.environ.get("SKIP_E1") == "1":
                nc.gpsimd.memset(out1[:], 1.0)
            else:
                edge_phase(table1, 1, ad1_all, emit1)

            # ------------- L2 node phase (octant-local) -------------
            for t in range(n_tiles):
                den = sp.tile([P, 8], f32, tag="den")
                nc.vector.tensor_copy(out=den[:], in_=out1[:, t * 80 + 64:t * 80 + 72])
                rec = sp.tile([P, 8], f32, tag="rec")
                nc.vector.reciprocal(rec[:], den[:])
                recb = sp.tile([P, 8], b16, tag="recb")
                nc.vector.tensor_copy(out=recb[:], in_=rec[:])
                h1f = sp.tile([P, 64], f32, tag="h1f")
                nc.vector.tensor_tensor(
                    out=h1f[:].rearrange("p (h c) -> p h c", c=8),
                    in0=out1[:, t * 80:t * 80 + 64].rearrange("p (h c) -> p h c", c=8),
                    in1=recb[:].unsqueeze(2).broadcast_to([P, 8, 8]), op=OP.mult)
                nc.vector.tensor_tensor(out=h1f[:], in0=h1f[:], in1=b1b[:], op=OP.add)
                # ELU: out = max(x,0) + exp(min(x,0)) - 1
                xm = sp.tile([P, 64], f32, tag="xm")
                nc.vector.tensor_scalar(out=xm[:], in0=h1f[:], scalar1=0.0,
                                        scalar2=None, op0=OP.min)
                xe = sp.tile([P, 64], f32, tag="xe")
                nc.scalar.activation(out=xe[:], in_=xm[:], func=AF.Exp)
                xp = sp.tile([P, 64], b16, tag="xp")
                nc.vector.tensor_scalar(out=xp[:], in0=h1f[:], scalar1=0.0,
                                        scalar2=None, op0=OP.max)
                h1e = sp.tile([P, 64], b16, tag="h1e")
                nc.vector.tensor_scalar(out=h1e[:], in0=xe[:], scalar1=-1.0,
                                        scalar2=None, op0=OP.add, accum_out=None)
                nc.vector.tensor_tensor(out=h1e[:], in0=h1e[:], in1=xp[:], op=OP.add)
                # transpose h1e -> lhsT [64, 128]
                trp = ptr.tile([P, P], b16, space="PSUM", tag="tr")
                nc.tensor.transpose(out=trp[:64, :], in_=h1e[:], identity=ident[:])
                h1t = sp.tile([64, P], b16, tag="h1t")
                nc.scalar.activation(out=h1t[:], in_=trp[:64, :], func=AF.Copy)
                ps2 = pl2.tile([P, 80], f32, space="PSUM", tag="l2")
                nc.tensor.matmul(out=ps2[:, 0:66], lhsT=h1t[:], rhs=w2ext[:],
                                 start=True, stop=True)
                st2 = nsp.tile([P, 66], b16, tag="st2")
                nc.scalar.activation(out=st2[:], in_=ps2[:, 0:66], func=AF.Copy)
                nc.sync.dma_start(out=h2loc[t * P:(t + 1) * P, 0:66], in_=st2[:])

            # ------------- AllGather h2 octants -> table2 -------------
            import os as _os
            if _os.environ.get("SKIP_CC") == "1":
                for k in range(n_cores):
                    nc.sync.dma_start(
                        out=table2[1 + k * per_core:1 + (k + 1) * per_core, :],
                        in_=h2loc[:])
            else:
                nc.gpsimd.collective_compute(
                    "AllGather", mybir.AluOpType.bypass,
                    replica_groups=[list(range(n_cores))],
                    ins=[h2loc[:].opt()],
                    outs=[table2[1:1 + Npad, :].opt()],
                )

            # ------------- L2 -------------
            # alpha_d2 = col 65 of table2 rows (written by L2 node phase)
            ad2_all = load_alpha_d(table2, -7, 1)   # 72 + (-7) = 65

            def emit2(t, acc):
                rec2 = sp.tile([P, 1], f32, tag="rec2")
                dn2 = sp.tile([P, 1], f32, tag="dn2")
                nc.vector.tensor_scalar(out=dn2[:], in0=acc[:, 64:65], scalar1=1e-16,
                                        scalar2=None, op0=OP.add)
                nc.vector.reciprocal(rec2[:], dn2[:])
                o2 = sp.tile([P, 64], f32, tag="o2")
                nc.vector.tensor_scalar(out=o2[:], in0=acc[:, 0:64], scalar1=rec2[:],
                                        scalar2=None, op0=OP.mult)
                nc.vector.tensor_tensor(out=o2[:], in0=o2[:], in1=b2b[:], op=OP.add)
                m = sp.tile([P, 1], f32, tag="m")
                nc.vector.tensor_reduce(out=m[:], in_=o2[:], op=OP.max,
                                        axis=mybir.AxisListType.X)
                negm = sp.tile([P, 1], f32, tag="negm")
                nc.vector.tensor_scalar(out=negm[:], in0=m[:], scalar1=-1.0,
                                        scalar2=None, op0=OP.mult)
                scr = sp.tile([P, 64], f32, tag="scr")
                sume = sp.tile([P, 1], f32, tag="sume")
                nc.scalar.activation(out=scr[:], in_=o2[:], func=AF.Exp,
                                     bias=negm[:], accum_out=sume[:])
                lns = sp.tile([P, 1], f32, tag="lns")
                nc.scalar.activation(out=lns[:], in_=sume[:], func=AF.Ln)
                res = sp.tile([P, 64], f32, tag="res")
                nc.vector.tensor_scalar(out=res[:], in0=o2[:], scalar1=m[:],
                                        scalar2=lns[:], op0=OP.subtract,
                                        op1=OP.subtract)
                nc.sync.dma_start(out=OUT[t * P:(t + 1) * P, :], in_=res[:])

            if _os2.environ.get("SKIP_E2") == "1":
                zres = sp.tile([P, 64], f32, tag="zres")
                nc.gpsimd.memset(zres[:], 0.0)
                for t in range(n_tiles):
                    nc.sync.dma_start(out=OUT[t * P:(t + 1) * P, :], in_=zres[:])
            else:
                edge_phase(table2, 2, ad2_all, emit2)

    nc.compile()
    return nc


# ----------------------------------------------------------------------------
# Host entry point
# ----------------------------------------------------------------------------
def _make_inputs(inputs, meta, idx16, node_of_pos):
    N = meta["N"]; Npad = meta["Npad"]; n_cores = meta["n_cores"]
    x = np.asarray(inputs["x"], dtype=np.float32)
    xp = np.zeros((Npad, F_IN), dtype=np.float32)
    valid = node_of_pos >= 0
    xp[valid] = x[node_of_pos[valid]]
    xT = np.ascontiguousarray(xp.T).astype(bf16)

    W1 = np.asarray(inputs["W1"], dtype=np.float32)
    a_s1 = np.asarray(inputs["a_src1"], dtype=np.float32)
    a_d1 = np.asarray(inputs["a_dst1"], dtype=np.float32)
    A1 = np.zeros((64, 16), dtype=np.float32)
    for h in range(H1):
        A1[h * C1:(h + 1) * C1, h] = a_s1[h]
        A1[h * C1:(h + 1) * C1, 8 + h] = a_d1[h]
    W2 = np.asarray(inputs["W2"], dtype=np.float32)
    a_s2 = np.asarray(inputs["a_src2"], dtype=np.float32).reshape(C2, 1)
    a_d2 = np.asarray(inputs["a_dst2"], dtype=np.float32).reshape(C2, 1)
    A2 = np.concatenate([a_s2, a_d2], axis=1)
    common = dict(
        xT=xT, W1=W1, W1T=np.ascontiguousarray(W1.T), A1=A1,
        W2=W2, W2T=np.ascontiguousarray(W2.T), A2=A2,
        B1=np.asarray(inputs["b1"], np.float32).reshape(1, 64),
        B2=np.asarray(inputs["b2"], np.float32).reshape(1, C2),
    )
    return [dict(common, IDX=np.ascontiguousarray(idx16[k])) for k in range(n_cores)]


def kernel(**inputs):
    x = np.asarray(inputs["x"])
    edge_index = np.asarray(inputs["edge_index"])
    N = x.shape[0]
    meta, idx16, node_of_pos = host_prep(edge_index, N, NCORES)
    nc = build_kernel(meta)
    in_maps = _make_inputs(inputs, meta, idx16, node_of_pos)
    res = run_bass_kernel_spmd(nc, in_maps, list(range(NCORES)))
    out = np.empty((N, C2), dtype=np.float32)
    for k in range(NCORES):
        o = res.results[k]["OUT"]
        pos0 = k * meta["per_core"]
        nodes = node_of_pos[pos0:pos0 + meta["per_core"]]
        valid = nodes >= 0
        out[nodes[valid]] = o[valid.nonzero()[0]]
    return out



# revision 5
# speedup vs baseline: 4.0885x; 4.0885x over previous
"""Self-contained Trainium2 Bass kernel for the 2-layer GAT (nn_GAT_6451040878848).

Sharding: nodes are sorted by exact in-degree and dealt to 8 cores with a
per-run assignment (LAP) that balances every dst's in-neighborhood across the
two overlapping int16 gather banks; flex edges (srcs reachable from either
bank) are split per 128-dst tile to minimize total slot count. Core k owns a
contiguous `per_core`-row octant of the permuted node table and aggregates
every edge whose dst is in its octant (dst-octant edge sharding -> no
all-reduce). The only collective is one bf16 AllGather of the layer-2 node
table. Layer-1 node phase is replicated from a host-transposed bf16 copy of x.

Edge phase: per 128-dst tile, edges are slot-major (slot c = c-th in-edge of
each dst; dst == partition), gathered from the node table with dma_gather
(int16 idx, two 65536-row banks via biased bases), weights w =
exp(leaky_relu(alpha_s[src]+alpha_d[dst])) (no max-subtraction: logits are in
[-1,3]), and scattered by an identity-lhsT matmul accumulating in PSUM; the
softmax denominator rides along as extra rhs columns.
"""
import numpy as np
import ml_dtypes

import concourse.bacc as bacc
import concourse.bass as bass
import concourse.tile as tile
from concourse import mybir
from concourse.bass_utils import run_bass_kernel_spmd

P = 128
TROW = 128           # bf16 elements per node-table row (256B)
NEG_SLOPE = 0.2
F_IN = 128
H1, C1 = 8, 8
C2 = 64
BANK_ROWS = 65536
DEG_QUANT = 4
NCORES = 8

bf16 = ml_dtypes.bfloat16


# ----------------------------------------------------------------------------
# Host-side graph preprocessing (integer/index work only)
# ----------------------------------------------------------------------------
def host_prep(edge_index: np.ndarray, n_nodes: int, n_cores: int = NCORES):
    N = n_nodes
    loops = np.arange(N, dtype=np.int64)
    src = np.concatenate([edge_index[0].astype(np.int64), loops])
    dst = np.concatenate([edge_index[1].astype(np.int64), loops])

    deg = np.bincount(dst, minlength=N)

    per_core = -(-N // (n_cores * P)) * P          # 128-aligned octant size
    Npad = per_core * n_cores
    n_tiles = per_core // P
    rows_total = Npad + 3                          # 0=padlow, 1..Npad nodes(+dummies), Npad+1=padhigh
    single_bank = rows_total <= 32768

    if single_bank:
        degq = -(-deg // DEG_QUANT) * DEG_QUANT
        rank = np.argsort(degq, kind="stable")
        node_rank = np.empty(N, dtype=np.int64)
        node_rank[rank] = np.arange(N)
        core_of = node_rank % n_cores
        local_of = node_rank // n_cores
    else:
        # Balanced zone coloring: sort nodes by exact in-degree (tiles get
        # uniform total counts), then within each 8-rank run choose which
        # node goes to which core so every dst's in-neighborhood splits
        # proportionally across the three int16-bank zones:
        #   zone 0 rows [1, LO_END)        reachable only via the low bank
        #   zone 1 rows [LO_END, HI_START) reachable via BOTH banks (flex)
        #   zone 2 rows [HI_START, ...)    reachable only via the high bank
        from scipy.optimize import linear_sum_assignment
        LO_END = rows_total - BANK_ROWS
        HI_START = BANK_ROWS
        order0 = np.argsort(deg, kind="stable")            # rank -> node
        so_ = np.argsort(src, kind="stable")
        dst_by_src = dst[so_]
        indptr = np.zeros(N + 1, np.int64)
        np.cumsum(np.bincount(src, minlength=N), out=indptr[1:])

        def zones_for(j):
            r = 1 + np.arange(n_cores) * per_core + j
            return np.where(r < LO_END, 0, np.where(r < HI_START, 1, 2))

        allz = np.concatenate([zones_for(j) for j in range(per_core)])
        pz = np.bincount(allz, minlength=3) / len(allz)
        target = deg[:, None] * pz[None, :]
        cnt = np.zeros((N, 3), np.float64)
        core_of_rank = np.full(Npad, -1, np.int64)
        zcache = [zones_for(j) for j in range(per_core)]

        for sweep in range(3):
            for j in range(per_core):
                r0 = 8 * j
                nreal = min(8, N - r0)
                if nreal <= 0:
                    continue
                nodes = order0[r0:r0 + nreal]
                zk = zcache[j]
                G = np.empty((nreal, 3), np.float64)
                for i, u in enumerate(nodes):
                    w = dst_by_src[indptr[u]:indptr[u + 1]]
                    if sweep > 0:
                        np.subtract.at(cnt, (w, zk[core_of_rank[r0 + i]]), 1)
                    G[i] = (target[w] - cnt[w]).sum(axis=0)
                ri, ci = linear_sum_assignment(-G[:, zk])
                for i, k in zip(ri, ci):
                    core_of_rank[r0 + i] = k
                    u = nodes[i]
                    w = dst_by_src[indptr[u]:indptr[u + 1]]
                    np.add.at(cnt, (w, zk[k]), 1)

        ranks = np.arange(N)
        core_of = np.empty(N, np.int64)
        local_of = np.empty(N, np.int64)
        core_of[order0] = core_of_rank[ranks]
        local_of[order0] = ranks // 8

    pos = core_of * per_core + local_of
    row = 1 + pos
    node_of_pos = np.full(Npad, -1, dtype=np.int64)
    node_of_pos[pos] = np.arange(N)
    assert (node_of_pos >= 0).sum() == N

    pad_low_row = 0
    pad_high_row = Npad + 1
    base_low = 0 if single_bank else BANK_ROWS // 2
    base_high = 0 if single_bank else (rows_total - BANK_ROWS + BANK_ROWS // 2)
    if not single_bank:
        assert rows_total <= 2 * BANK_ROWS, "need more than 2 banks"

    e_core = core_of[dst]
    e_tile = local_of[dst] // P
    e_part = local_of[dst] % P
    e_srcrow = row[src]

    if single_bank:
        e_low = np.ones(len(src), bool)
        cnt_low = np.zeros((n_cores, n_tiles, P), dtype=np.int32)
        cnt_high = np.zeros((n_cores, n_tiles, P), dtype=np.int32)
        np.add.at(cnt_low, (e_core, e_tile, e_part), 1)
        D_low = cnt_low.max(axis=(0, 2)).astype(np.int64)
        D_high = cnt_high.max(axis=(0, 2)).astype(np.int64)
        extra = 0
        slots_low = D_low + extra
    else:
        # per-cell zone counts; per-tile minimal (L, H); flex edges split so
        # every cell's low-bank count <= L and high-bank count <= H.
        LO_END = rows_total - BANK_ROWS
        HI_START = BANK_ROWS
        ez = np.where(e_srcrow < LO_END, 0, np.where(e_srcrow < HI_START, 1, 2))
        cnt3 = np.zeros((3, n_cores, n_tiles, P), np.int32)
        np.add.at(cnt3, (ez, e_core, e_tile, e_part), 1)
        lo_only = cnt3[0]; flex = cnt3[1]; hi_only = cnt3[2]
        tot = lo_only + flex + hi_only
        D_low = np.zeros(n_tiles, np.int64)
        D_high = np.zeros(n_tiles, np.int64)
        for t in range(n_tiles):
            lo = lo_only[:, t, :].ravel(); fl = flex[:, t, :].ravel()
            to = tot[:, t, :].ravel(); hi = hi_only[:, t, :].ravel()
            Lmin = int(lo.max()); Hfloor = int(hi.max())
            best = None
            for L in range(Lmin, Lmin + 64):
                H = max(Hfloor, int((to - np.minimum(lo + fl, L)).max()))
                if best is None or L + H < best[0] + best[1]:
                    best = (L, H)
                if H == Hfloor:
                    break
            D_low[t], D_high[t] = best
        # per-cell low capacity
        Lb = D_low[None, :, None].astype(np.int64)
        Hb = D_high[None, :, None].astype(np.int64)
        low_cap = np.clip(tot - Hb, lo_only, np.minimum(lo_only + flex, Lb))
        assert (low_cap >= lo_only).all() and (tot - low_cap <= Hb).all() \
            and (low_cap <= Lb).all()
        extra = 1    # +1 throwaway slot terminating the low idx list positively
        slots_low = D_low + extra

    # ---- slot assignment ----
    if single_bank:
        order = np.lexsort((e_part, e_tile, e_core))
        so_core = e_core[order]; so_tile = e_tile[order]; so_part = e_part[order]
        so_row = e_srcrow[order]
        grp = (so_core * n_tiles + so_tile) * P + so_part
        _, inv, counts = np.unique(grp, return_inverse=True, return_counts=True)
        starts = np.concatenate([[0], np.cumsum(counts)[:-1]])
        slot_in_bank = np.arange(len(grp)) - starts[inv]
        so_low = np.ones(len(grp), bool)
    else:
        # within-cell order (zone, row): the high list ends on its largest
        # index, so trailing stream entries are as positive as possible
        order = np.lexsort((e_srcrow, ez, e_part, e_tile, e_core))
        so_core = e_core[order]; so_tile = e_tile[order]; so_part = e_part[order]
        so_row = e_srcrow[order]
        grp = (so_core * n_tiles + so_tile) * P + so_part
        _, inv, counts = np.unique(grp, return_inverse=True, return_counts=True)
        starts = np.concatenate([[0], np.cumsum(counts)[:-1]])
        pos_in_cell = np.arange(len(grp)) - starts[inv]
        cap = low_cap[so_core, so_tile, so_part]
        so_low = pos_in_cell < cap
        slot_in_bank = np.where(so_low, pos_in_cell, pos_in_cell - cap)

        # a tile whose high idx stream could end on a negative idx (some cell
        # completely full AND its largest-row high edge below base_high) gets
        # one extra all-pad high slot so the stream ends positive (pads have
        # idx 32766); trailing negative idxs are skipped by the gather ucode.
        maxrow_high = np.zeros((n_cores, n_tiles, P), np.int64)
        hi_m = ~so_low
        np.maximum.at(maxrow_high,
                      (so_core[hi_m], so_tile[hi_m], so_part[hi_m]), so_row[hi_m])
        high_cnt = tot - low_cap
        for t in range(n_tiles):
            if D_high[t] == 0:
                continue
            full = high_cnt[:, t, :] == D_high[t]
            if full.any() and (maxrow_high[:, t, :][full] < base_high).any():
                D_high[t] += 1

    max_slots = int((slots_low + D_high).max()) if n_tiles else 0
    srows = np.full((n_cores, n_tiles, max_slots, P), -1, dtype=np.int64)
    slot_abs = np.where(so_low, slot_in_bank, slots_low[so_tile] + slot_in_bank)
    srows[so_core, so_tile, slot_abs, so_part] = so_row

    for t in range(n_tiles):
        dl = int(D_low[t]); sl = int(slots_low[t]); dh = int(D_high[t])
        blk = srows[:, t, :dl, :]
        blk[blk < 0] = pad_low_row
        srows[:, t, :dl, :] = blk
        if not single_bank:
            srows[:, t, dl, :] = base_low          # throwaway slot (idx 0, positive)
            if dh:
                blk2 = srows[:, t, sl:sl + dh, :]
                blk2[blk2 < 0] = pad_high_row
                srows[:, t, sl:sl + dh, :] = blk2

    # ---- int16 idx arrays ----
    col_off = []
    cols = 0
    for t in range(n_tiles):
        lo = cols; cols += int(slots_low[t]) * 8
        hi = cols; cols += int(D_high[t]) * 8
        col_off.append((lo, hi))
    idx16 = np.zeros((n_cores, 16, max(cols, 1)), dtype=np.int16)
    for t in range(n_tiles):
        sl = int(slots_low[t]); dh = int(D_high[t])
        lo, hi = col_off[t]
        jl = srows[:, t, :sl, :].reshape(n_cores, -1) - base_low
        assert jl.max() <= 32767 and jl.min() >= -32768
        idx16[:, :, lo:lo + sl * 8] = jl.astype(np.int16).reshape(n_cores, -1, 16).transpose(0, 2, 1)
        if dh:
            jh = srows[:, t, sl:sl + dh, :].reshape(n_cores, -1) - base_high
            assert jh.max() <= 32767 and jh.min() >= -32768
            idx16[:, :, hi:hi + dh * 8] = jh.astype(np.int16).reshape(n_cores, -1, 16).transpose(0, 2, 1)
    idx16 = np.tile(idx16, (1, 8, 1))

    meta = dict(
        N=N, Npad=Npad, n_cores=n_cores, per_core=per_core, n_tiles=n_tiles,
        rows_total=rows_total, single_bank=single_bank,
        base_low=int(base_low), base_high=int(base_high),
        slots_low=[int(v) for v in slots_low],
        D_low=[int(v) for v in D_low],
        D_high=[int(v) for v in D_high],
        col_off=col_off, idx_cols=int(max(cols, 1)),
    )
    return meta, idx16, node_of_pos


# ----------------------------------------------------------------------------
# Device kernel
# ----------------------------------------------------------------------------
def build_kernel(meta):
    Npad = meta["Npad"]; n_cores = meta["n_cores"]; per_core = meta["per_core"]
    n_tiles = meta["n_tiles"]; rows_total = meta["rows_total"]
    slots_low = meta["slots_low"]; D_low = meta["D_low"]; D_high = meta["D_high"]
    col_off = meta["col_off"]; idx_cols = meta["idx_cols"]
    single_bank = meta["single_bank"]
    base_low = meta["base_low"]; base_high = meta["base_high"]
    n_groups = Npad // (4 * P)

    nc = bacc.Bacc("TRN2", target_bir_lowering=False, debug=False,
                   num_devices=n_cores, num_swdge_queues=4)
    f32, b16, i16 = mybir.dt.float32, mybir.dt.bfloat16, mybir.dt.int16
    AF = mybir.ActivationFunctionType
    OP = mybir.AluOpType

    xT = nc.dram_tensor("xT", [F_IN, Npad], b16, kind="ExternalInput").ap()
    W1 = nc.dram_tensor("W1", [F_IN, 64], f32, kind="ExternalInput").ap()
    W1T = nc.dram_tensor("W1T", [64, F_IN], f32, kind="ExternalInput").ap()
    A1 = nc.dram_tensor("A1", [64, 16], f32, kind="ExternalInput").ap()
    W2 = nc.dram_tensor("W2", [64, C2], f32, kind="ExternalInput").ap()
    W2T = nc.dram_tensor("W2T", [C2, 64], f32, kind="ExternalInput").ap()
    A2 = nc.dram_tensor("A2", [C2, 2], f32, kind="ExternalInput").ap()
    B1 = nc.dram_tensor("B1", [1, 64], f32, kind="ExternalInput").ap()
    B2 = nc.dram_tensor("B2", [1, C2], f32, kind="ExternalInput").ap()
    IDX = nc.dram_tensor("IDX", [P, idx_cols], i16, kind="ExternalInput").ap()
    OUT = nc.dram_tensor("OUT", [per_core, C2], f32, kind="ExternalOutput").ap()

    with tile.TileContext(nc) as tc:
        with tc.tile_pool(name="dram", bufs=1, space="DRAM") as dram, \
             tc.tile_pool(name="consts", bufs=1) as cp, \
             tc.tile_pool(name="stg", bufs=3) as nsp, \
             tc.tile_pool(name="xtp", bufs=3) as xtp, \
             tc.tile_pool(name="gpl", bufs=3) as gp, \
             tc.tile_pool(name="idxp", bufs=3) as idxp, \
             tc.tile_pool(name="vwp", bufs=3) as vwp, \
             tc.tile_pool(name="stat", bufs=4) as sp, \
             tc.tile_pool(name="pacc", bufs=2, space="PSUM") as pacc, \
             tc.tile_pool(name="pnode", bufs=2, space="PSUM") as pnode, \
             tc.tile_pool(name="ptr", bufs=2, space="PSUM") as ptr, \
             tc.tile_pool(name="pl2", bufs=2, space="PSUM") as pl2:

            table1 = dram.tile([rows_total, TROW], b16)
            table2 = dram.tile([rows_total, TROW], b16)
            h2loc = dram.tile([per_core, TROW], b16)

            # ------------- constants -------------
            ident = cp.tile([P, P], b16)
            nc.gpsimd.memset(ident[:], 0.0)
            iota_i = cp.tile([P, 1], mybir.dt.int32)
            nc.gpsimd.iota(iota_i[:], pattern=[[0, 1]], base=0, channel_multiplier=1)
            iota_f = cp.tile([P, 1], f32)
            nc.vector.tensor_copy(out=iota_f[:], in_=iota_i[:])
            iotar_i = cp.tile([P, P], mybir.dt.int32)
            nc.gpsimd.iota(iotar_i[:], pattern=[[1, P]], base=0, channel_multiplier=0)
            iotar_f = cp.tile([P, P], f32)
            nc.vector.tensor_copy(out=iotar_f[:], in_=iotar_i[:])
            nc.vector.tensor_scalar(out=ident[:], in0=iotar_f[:], scalar1=iota_f[:],
                                    scalar2=None, op0=OP.is_equal)

            w1f = cp.tile([P, 64], f32)
            nc.sync.dma_start(out=w1f[:], in_=W1)
            w1t = cp.tile([64, P], f32)
            nc.sync.dma_start(out=w1t[:], in_=W1T)
            a1t = cp.tile([64, 16], f32)
            nc.sync.dma_start(out=a1t[:], in_=A1)
            w2f = cp.tile([64, 64], f32)
            nc.sync.dma_start(out=w2f[:], in_=W2)
            w2t = cp.tile([64, 64], f32)
            nc.sync.dma_start(out=w2t[:], in_=W2T)
            a2t = cp.tile([64, 2], f32)
            nc.sync.dma_start(out=a2t[:], in_=A2)
            b1r = cp.tile([1, 64], f32)
            nc.sync.dma_start(out=b1r[:], in_=B1)
            b1b = cp.tile([P, 64], f32)
            nc.gpsimd.partition_broadcast(b1b[:], b1r[:])
            b2r = cp.tile([1, 64], f32)
            nc.sync.dma_start(out=b2r[:], in_=B2)
            b2b = cp.tile([P, 64], f32)
            nc.gpsimd.partition_broadcast(b2b[:], b2r[:])

            wext1 = cp.tile([P, 80], b16)
            ws_ps = pnode.tile([P, 320], f32, space="PSUM", tag="np")
            nc.tensor.matmul(out=ws_ps[:, 0:16], lhsT=w1t[:], rhs=a1t[:], start=True, stop=True)
            nc.vector.tensor_copy(out=wext1[:, 0:64], in_=w1f[:])
            nc.vector.tensor_copy(out=wext1[:, 64:80], in_=ws_ps[:, 0:16])

            w2ext = cp.tile([64, 66], b16)
            ws2_ps = pnode.tile([P, 320], f32, space="PSUM", tag="np")
            nc.tensor.matmul(out=ws2_ps[:64, 0:2], lhsT=w2t[:], rhs=a2t[:], start=True, stop=True)
            nc.vector.tensor_copy(out=w2ext[:, 0:64], in_=w2f[:])
            nc.vector.tensor_copy(out=w2ext[:, 64:66], in_=ws2_ps[:64, 0:2])

            padrow = cp.tile([1, TROW], b16)
            nc.gpsimd.memset(padrow[:], 0.0)
            nc.gpsimd.memset(padrow[:, 64:72], -40.0)
            for tbl in (table1, table2):
                nc.sync.dma_start(out=tbl[0:1, :], in_=padrow[:])
                nc.sync.dma_start(out=tbl[Npad + 1:Npad + 2, :], in_=padrow[:])

            # core id register (for alpha-d SBUF slicing)
            pid = nc.partition_id()

            # ------------- L1 node phase (replicated) -------------
            for g in range(n_groups):
                base = g * 4 * P
                xt = xtp.tile([P, 4 * P], b16, tag="xt")
                nc.sync.dma_start(out=xt[:], in_=xT[:, base:base + 4 * P])
                ps = pnode.tile([P, 320], f32, space="PSUM", tag="np")
                stage = nsp.tile([P, 4 * 80], b16, tag="stage")
                for s in range(4):
                    nc.tensor.matmul(out=ps[:, s * 80:(s + 1) * 80],
                                     lhsT=xt[:, s * P:(s + 1) * P],
                                     rhs=wext1[:], start=True, stop=True)
                nc.scalar.activation(
                    out=stage[:], in_=ps[:], func=AF.Copy)
                nc.sync.dma_start(
                    out=table1[1 + base:1 + base + 4 * P, 0:80].rearrange("(s p) r -> p s r", p=P),
                    in_=stage[:].rearrange("p (s r) -> p s r", r=80))

            # ------------- alpha-d preload helper -------------
            def load_alpha_d(table, cols, width):
                """[128, n_cores * n_tiles * width] from table[row, 64+cols...]"""
                t_all = cp.tile([P, n_cores * n_tiles * width], b16,
                                tag=f"ad{cols}")
                for k in range(n_cores):
                    r0 = 1 + k * per_core
                    nc.sync.dma_start(
                        out=t_all[:, k * n_tiles * width:(k + 1) * n_tiles * width]
                            .rearrange("p (t w) -> p t w", w=width),
                        in_=table[r0:r0 + per_core, 72 + cols:72 + cols + width]
                            .rearrange("(t p) w -> p t w", p=P))
                return t_all

            # ------------- edge phase -------------
            def edge_phase(table, layer, ad_all, emit):
                """layer 1: heads=8, vw=72; layer 2: heads=1, vw=65."""
                heads = H1 if layer == 1 else 1
                vw_w = 72 if layer == 1 else 65
                if single_bank:
                    in_lo, in_hi = table[:, :], None
                else:
                    in_lo = table[base_low:min(base_low + BANK_ROWS, rows_total), :]
                    in_hi = table[base_high:rows_total, :]
                for t in range(n_tiles):
                    sl = slots_low[t]; dl = D_low[t]; dh = D_high[t]
                    stot = sl + dh
                    lo_c, hi_c = col_off[t]
                    idxt = idxp.tile([P, (sl + dh) * 8], i16, tag="idx")
                    nc.sync.dma_start(out=idxt[:], in_=IDX[:, lo_c:lo_c + (sl + dh) * 8])
                    G = gp.tile([P, stot * TROW], b16, tag="G")
                    nc.gpsimd.dma_gather(
                        out_ap=G[:, 0:sl * TROW].rearrange("p (s r) -> p s r", r=TROW),
                        in_ap=in_lo, idxs_ap=idxt[:, 0:sl * 8],
                        num_idxs=sl * P, num_idxs_reg=sl * P, elem_size=TROW,
                        queue_num=(2 * t) % 4, single_packet=False)
                    if dh:
                        nc.gpsimd.dma_gather(
                            out_ap=G[:, sl * TROW:stot * TROW].rearrange("p (s r) -> p s r", r=TROW),
                            in_ap=in_hi, idxs_ap=idxt[:, sl * 8:(sl + dh) * 8],
                            num_idxs=dh * P, num_idxs_reg=dh * P, elem_size=TROW,
                            queue_num=(2 * t + 1) % 4, single_packet=False)

                    # real slots: [0,dl) and [sl, sl+dh)
                    Gv = G[:].rearrange("p (s r) -> p s r", r=TROW)
                    adw = heads
                    ad_off = pid * (n_tiles * adw) + t * adw
                    ad_t = sp.tile([P, adw], b16, tag="adt")
                    nc.vector.tensor_copy(out=ad_t[:], in_=ad_all[:, bass.ds(ad_off, adw)])
                    w_all = sp.tile([P, stot * 8], f32, tag="wf")
                    wb_all = sp.tile([P, stot * 8], b16, tag="wb")
                    for (s0, ns) in ((0, dl), (sl, dh)) if dh else ((0, dl),):
                        if ns == 0:
                            continue
                        als = Gv[:, s0:s0 + ns, 64:64 + heads]
                        e_t = sp.tile([P, stot * 8], f32, tag="et")
                        if layer == 1:
                            adv = ad_t[:].unsqueeze(1).broadcast_to([P, ns, 8])
                            nc.vector.tensor_tensor(
                                out=e_t[:, s0 * 8:(s0 + ns) * 8].rearrange("p (s h) -> p s h", h=8),
                                in0=als, in1=adv, op=OP.add)
                            nc.scalar.activation(
                                out=e_t[:, s0 * 8:(s0 + ns) * 8],
                                in_=e_t[:, s0 * 8:(s0 + ns) * 8],
                                func=AF.Prelu, alpha=NEG_SLOPE)
                            nc.scalar.activation(
                                out=w_all[:, s0 * 8:(s0 + ns) * 8],
                                in_=e_t[:, s0 * 8:(s0 + ns) * 8], func=AF.Exp)
                            nc.vector.tensor_copy(
                                out=wb_all[:, s0 * 8:(s0 + ns) * 8],
                                in_=w_all[:, s0 * 8:(s0 + ns) * 8])
                        else:
                            adv = ad_t[:]
                            nc.scalar.activation(
                                out=e_t[:, s0:s0 + ns],
                                in_=als.rearrange("p s one -> p (s one)"),
                                func=AF.Prelu, bias=adv, alpha=NEG_SLOPE)
                            nc.scalar.activation(
                                out=w_all[:, s0:s0 + ns],
                                in_=e_t[:, s0:s0 + ns], func=AF.Exp)
                            nc.vector.tensor_copy(
                                out=wb_all[:, s0:s0 + ns],
                                in_=w_all[:, s0:s0 + ns])

                    Vw = vwp.tile([P, stot * vw_w], b16, tag="vw")
                    Vv = Vw[:].rearrange("p (s c) -> p s c", c=vw_w)
                    for (s0, ns) in ((0, dl), (sl, dh)) if dh else ((0, dl),):
                        if ns == 0:
                            continue
                        if layer == 1:
                            wbv = wb_all[:].rearrange("p (s h) -> p s h", h=8)[:, s0:s0 + ns, :] \
                                .unsqueeze(3).broadcast_to([P, ns, 8, 8])
                            nc.vector.tensor_tensor(
                                out=Vv[:, s0:s0 + ns, 0:64].rearrange("p s (h c) -> p s h c", c=8),
                                in0=Gv[:, s0:s0 + ns, 0:64].rearrange("p s (h c) -> p s h c", c=8),
                                in1=wbv, op=OP.mult)
                            nc.vector.tensor_copy(
                                out=Vv[:, s0:s0 + ns, 64:72],
                                in_=wb_all[:].rearrange("p (s h) -> p s h", h=8)[:, s0:s0 + ns, :])
                        else:
                            wbv = wb_all[:, s0:s0 + ns].unsqueeze(2).broadcast_to([P, ns, 64])
                            nc.vector.tensor_tensor(
                                out=Vv[:, s0:s0 + ns, 0:64],
                                in0=Gv[:, s0:s0 + ns, 0:64], in1=wbv, op=OP.mult)
                            nc.vector.tensor_copy(
                                out=Vv[:, s0:s0 + ns, 64:65],
                                in_=wb_all[:, s0:s0 + ns].unsqueeze(2))

                    acc = pacc.tile([P, vw_w], f32, space="PSUM", tag="acc")
                    real = [s for s in range(dl)] + [s for s in range(sl, stot)]
                    for i, s in enumerate(real):
                        nc.tensor.matmul(out=acc[:], lhsT=ident[:],
                                         rhs=Vw[:, s * vw_w:(s + 1) * vw_w],
                                         start=(i == 0), stop=(i == len(real) - 1))
                    emit(t, acc)

            # ------------- L1 -------------
            import os as _os2
            ad1_all = load_alpha_d(table1, 0, 8)
            out1 = cp.tile([P, n_tiles * 80], b16)

            def emit1(t, acc):
                nc.vector.tensor_copy(out=out1[:, t * 80:t * 80 + 64], in_=acc[:, 0:64])
                nc.vector.tensor_scalar(out=out1[:, t * 80 + 64:t * 80 + 72],
                                        in0=acc[:, 64:72], scalar1=1e-16,
                                        scalar2=None, op0=OP.add)

            if _os2.environ.get("SKIP_E1") == "1":
                nc.gpsimd.memset(out1[:], 1.0)
            else:
                edge_phase(table1, 1, ad1_all, emit1)

            # ------------- L2 node phase (octant-local) -------------
            for t in range(n_tiles):
                den = sp.tile([P, 8], f32, tag="den")
                nc.vector.tensor_copy(out=den[:], in_=out1[:, t * 80 + 64:t * 80 + 72])
                rec = sp.tile([P, 8], f32, tag="rec")
                nc.vector.reciprocal(rec[:], den[:])
                recb = sp.tile([P, 8], b16, tag="recb")
                nc.vector.tensor_copy(out=recb[:], in_=rec[:])
                h1f = sp.tile([P, 64], f32, tag="h1f")
                nc.vector.tensor_tensor(
                    out=h1f[:].rearrange("p (h c) -> p h c", c=8),
                    in0=out1[:, t * 80:t * 80 + 64].rearrange("p (h c) -> p h c", c=8),
                    in1=recb[:].unsqueeze(2).broadcast_to([P, 8, 8]), op=OP.mult)
                nc.vector.tensor_tensor(out=h1f[:], in0=h1f[:], in1=b1b[:], op=OP.add)
                # ELU: out = max(x,0) + exp(min(x,0)) - 1
                xm = sp.tile([P, 64], f32, tag="xm")
                nc.vector.tensor_scalar(out=xm[:], in0=h1f[:], scalar1=0.0,
                                        scalar2=None, op0=OP.min)
                xe = sp.tile([P, 64], f32, tag="xe")
                nc.scalar.activation(out=xe[:], in_=xm[:], func=AF.Exp)
                xp = sp.tile([P, 64], b16, tag="xp")
                nc.vector.tensor_scalar(out=xp[:], in0=h1f[:], scalar1=0.0,
                                        scalar2=None, op0=OP.max)
                h1e = sp.tile([P, 64], b16, tag="h1e")
                nc.vector.tensor_scalar(out=h1e[:], in0=xe[:], scalar1=-1.0,
                                        scalar2=None, op0=OP.add, accum_out=None)
                nc.vector.tensor_tensor(out=h1e[:], in0=h1e[:], in1=xp[:], op=OP.add)
                # transpose h1e -> lhsT [64, 128]
                trp = ptr.tile([P, P], b16, space="PSUM", tag="tr")
                nc.tensor.transpose(out=trp[:64, :], in_=h1e[:], identity=ident[:])
                h1t = sp.tile([64, P], b16, tag="h1t")
                nc.scalar.activation(out=h1t[:], in_=trp[:64, :], func=AF.Copy)
                ps2 = pl2.tile([P, 80], f32, space="PSUM", tag="l2")
                nc.tensor.matmul(out=ps2[:, 0:66], lhsT=h1t[:], rhs=w2ext[:],
                                 start=True, stop=True)
                st2 = nsp.tile([P, 66], b16, tag="st2")
                nc.scalar.activation(out=st2[:], in_=ps2[:, 0:66], func=AF.Copy)
                nc.sync.dma_start(out=h2loc[t * P:(t + 1) * P, 0:66], in_=st2[:])

            # ------------- AllGather h2 octants -> table2 -------------
            import os as _os
            if _os.environ.get("SKIP_CC") == "1":
                for k in range(n_cores):
                    nc.sync.dma_start(
                        out=table2[1 + k * per_core:1 + (k + 1) * per_core, :],
                        in_=h2loc[:])
            else:
                nc.gpsimd.collective_compute(
                    "AllGather", mybir.AluOpType.bypass,
                    replica_groups=[list(range(n_cores))],
                    ins=[h2loc[:].opt()],
                    outs=[table2[1:1 + Npad, :].opt()],
                )

            # ------------- L2 -------------
            # alpha_d2 = col 65 of table2 rows (written by L2 node phase)
            ad2_all = load_alpha_d(table2, -7, 1)   # 72 + (-7) = 65

            def emit2(t, acc):
                rec2 = sp.tile([P, 1], f32, tag="rec2")
                dn2 = sp.tile([P, 1], f32, tag="dn2")
                nc.vector.tensor_scalar(out=dn2[:], in0=acc[:, 64:65], scalar1=1e-16,
                                        scalar2=None, op0=OP.add)
                nc.vector.reciprocal(rec2[:], dn2[:])
                o2 = sp.tile([P, 64], f32, tag="o2")
                nc.vector.tensor_scalar(out=o2[:], in0=acc[:, 0:64], scalar1=rec2[:],
                                        scalar2=None, op0=OP.mult)
                nc.vector.tensor_tensor(out=o2[:], in0=o2[:], in1=b2b[:], op=OP.add)
                m = sp.tile([P, 1], f32, tag="m")
                nc.vector.tensor_reduce(out=m[:], in_=o2[:], op=OP.max,
                                        axis=mybir.AxisListType.X)
                negm = sp.tile([P, 1], f32, tag="negm")
                nc.vector.tensor_scalar(out=negm[:], in0=m[:], scalar1=-1.0,
                                        scalar2=None, op0=OP.mult)
                scr = sp.tile([P, 64], f32, tag="scr")
                sume = sp.tile([P, 1], f32, tag="sume")
                nc.scalar.activation(out=scr[:], in_=o2[:], func=AF.Exp,
                                     bias=negm[:], accum_out=sume[:])
                lns = sp.tile([P, 1], f32, tag="lns")
                nc.scalar.activation(out=lns[:], in_=sume[:], func=AF.Ln)
                res = sp.tile([P, 64], f32, tag="res")
                nc.vector.tensor_scalar(out=res[:], in0=o2[:], scalar1=m[:],
                                        scalar2=lns[:], op0=OP.subtract,
                                        op1=OP.subtract)
                nc.sync.dma_start(out=OUT[t * P:(t + 1) * P, :], in_=res[:])

            if _os2.environ.get("SKIP_E2") == "1":
                zres = sp.tile([P, 64], f32, tag="zres")
                nc.gpsimd.memset(zres[:], 0.0)
                for t in range(n_tiles):
                    nc.sync.dma_start(out=OUT[t * P:(t + 1) * P, :], in_=zres[:])
            else:
                edge_phase(table2, 2, ad2_all, emit2)

    nc.compile()
    return nc


# ----------------------------------------------------------------------------
# Host entry point
# ----------------------------------------------------------------------------
def _make_inputs(inputs, meta, idx16, node_of_pos):
    N = meta["N"]; Npad = meta["Npad"]; n_cores = meta["n_cores"]
    x = np.asarray(inputs["x"], dtype=np.float32)
    xp = np.zeros((Npad, F_IN), dtype=np.float32)
    valid = node_of_pos >= 0
    xp[valid] = x[node_of_pos[valid]]
    xT = np.ascontiguousarray(xp.T).astype(bf16)

    W1 = np.asarray(inputs["W1"], dtype=np.float32)
    a_s1 = np.asarray(inputs["a_src1"], dtype=np.float32)
    a_d1 = np.asarray(inputs["a_dst1"], dtype=np.float32)
    A1 = np.zeros((64, 16), dtype=np.float32)
    for h in range(H1):
        A1[h * C1:(h + 1) * C1, h] = a_s1[h]
        A1[h * C1:(h + 1) * C1, 8 + h] = a_d1[h]
    W2 = np.asarray(inputs["W2"], dtype=np.float32)
    a_s2 = np.asarray(inputs["a_src2"], dtype=np.float32).reshape(C2, 1)
    a_d2 = np.asarray(inputs["a_dst2"], dtype=np.float32).reshape(C2, 1)
    A2 = np.concatenate([a_s2, a_d2], axis=1)
    common = dict(
        xT=xT, W1=W1, W1T=np.ascontiguousarray(W1.T), A1=A1,
        W2=W2, W2T=np.ascontiguousarray(W2.T), A2=A2,
        B1=np.asarray(inputs["b1"], np.float32).reshape(1, 64),
        B2=np.asarray(inputs["b2"], np.float32).reshape(1, C2),
    )
    return [dict(common, IDX=np.ascontiguousarray(idx16[k])) for k in range(n_cores)]


def kernel(**inputs):
    x = np.asarray(inputs["x"])
    edge_index = np.asarray(inputs["edge_index"])
    N = x.shape[0]
    meta, idx16, node_of_pos = host_prep(edge_index, N, NCORES)
    nc = build_kernel(meta)
    in_maps = _make_inputs(inputs, meta, idx16, node_of_pos)
    res = run_bass_kernel_spmd(nc, in_maps, list(range(NCORES)))
    out = np.empty((N, C2), dtype=np.float32)
    for k in range(NCORES):
        o = res.results[k]["OUT"]
        pos0 = k * meta["per_core"]
        nodes = node_of_pos[pos0:pos0 + meta["per_core"]]
        valid = nodes >= 0
        out[nodes[valid]] = o[valid.nonzero()[0]]
    return out

